# revision 34
# baseline (speedup 1.0000x reference)
"""MASA agent-attention kernel for Trainium2, 8-core SPMD.

Sharding: core = (batch b in 0..3) x (head-group hg in 0..1); each core
computes 4 heads (96 of 192 output channels) for one batch image.
No cross-core communication.

Per-core channel slabs of 128 (processed in this order):
  slab 0 "A": rows 0:96 = a-channels, rows 96:128 = v[0:32]
  slab 1 "K": rows 0:96 = k-channels, rows 96:128 = v[32:64]
  slab 2 "Q": rows 0:96 = q-channels, rows 96:128 = v[64:96]
so agent pooling (a) and the k-norm can start as early as possible, and
both norms use the same block-diagonal ones matrix at partition base 0.

v is transposed to [spatial, channel] with 3 large xbar-transpose DMAs
(out AP [128, chunk, ch] => out[p, c, ch] = v[ch, c*128+p]), replacing
the per-chunk transpose DMAs.

Engine-op partition windows must be 32-aligned and (base==0 or count<=32).
"""

import sys
import types
import numpy as np

import concourse.bacc as bacc
import concourse.bass as bass
import concourse.mybir as mybir
from concourse.tile import TileContext
from concourse.bass_utils import run_bass_kernel_spmd

F16 = mybir.dt.float16
F32 = mybir.dt.float32
AX = mybir.AxisListType
OP = mybir.AluOpType
AF = mybir.ActivationFunctionType

B, C, H, W = 4, 192, 128, 128
N = H * W              # 16384
M_AG = 64              # agent tokens
E_LAMBDA = 1e-4
RS = 130               # padded row stride for pre
PREFREE = RS * RS      # 16900
NB = 16                # blocks of 1024 spatial positions
BLK = 1024
NCH = 128              # k-side chunks of 128

# dwconv 3x3 tap split: PE does diag-matmul taps, DVE/GpSimd do
# scalar*tensor+tensor taps (dx=+-1 keeps their inner APs 4B-aligned).
# Slab A gives DVE only the psum-merge tap: DVE also runs the pooling
# reductions there (GpSimd cannot reduce along the free axis).
PE_TAPS = {s: [(-1, -1), (-1, 0), (-1, 1), (0, 0), (1, -1), (1, 0), (1, 1)]
           for s in range(3)}
DVE_TAPS = {0: [(0, -1)],           # first one also merges the PE psum
            1: [(0, -1)],
            2: [(0, -1)]}
GP_TAPS = {0: [(0, 1)],
           1: [(0, 1)],
           2: [(0, 1)]}
WDIAG_SLOT = {}
for _s in range(3):
    for _t in PE_TAPS[_s]:
        WDIAG_SLOT[(_s, _t[0], _t[1])] = len(WDIAG_SLOT)
NDIAG = len(WDIAG_SLOT)

# vT chunk layout (width 128): col 15 = 1, cols 16:112 = v[0:96], col 112 = 1
V0 = 16


def _tidx(dy, dx):
    return (dy + 1) * 3 + (dx + 1)


def _install_ntff_hook():
    try:
        import antenv.axon_hooks  # noqa: F401
        return
    except ImportError:
        pass
    try:
        from trn_agent_boot.trn_boot import _ntff_profile_via_ctypes
        hook = _ntff_profile_via_ctypes('/opt/axon/libaxon_pjrt.so')
        mod = types.ModuleType("antenv.axon_hooks")
        mod.get_axon_ntff_profile_hook = lambda: hook
        mod.set_axon_ntff_profile_hook = lambda h: None
        sys.modules["antenv.axon_hooks"] = mod
    except Exception:
        pass


def build_nc():
    nc = bacc.Bacc("TRN2", target_bir_lowering=False, debug=False, num_devices=8)

    # ---- DRAM I/O ----
    xin = nc.dram_tensor("xin", [192, N], F16, kind="ExternalInput").ap()
    w1a = nc.dram_tensor("w1a", [96, 384], F16, kind="ExternalInput").ap()
    w1b = nc.dram_tensor("w1b", [96, 384], F16, kind="ExternalInput").ap()
    wdiag = nc.dram_tensor("wdiag", [128, NDIAG * 128], F16, kind="ExternalInput").ap()
    wtap = nc.dram_tensor("wtap", [128, 27], F32, kind="ExternalInput").ap()
    tmp01 = nc.dram_tensor("tmp01", [96, 1], F32, kind="ExternalInput").ap()
    pat = nc.dram_tensor("pat", [128, 242], F16, kind="ExternalInput").ap()
    out_d = nc.dram_tensor("out", [96, N], F16, kind="ExternalOutput").ap()

    # ---- persistent SBUF ----
    scratch = nc.alloc_sbuf_tensor("scratch", [128, PREFREE], F16).ap()
    dwA = nc.alloc_sbuf_tensor("dwA", [128, N], F16).ap()
    dwK = nc.alloc_sbuf_tensor("dwK", [128, N], F16).ap()
    dwQ = nc.alloc_sbuf_tensor("dwQ", [128, N], F16).ap()
    vT = nc.alloc_sbuf_tensor("vT", [128, N], F16).ap()
    dws = [dwA, dwK, dwQ]
    w1a_s = nc.alloc_sbuf_tensor("w1a_s", [96, 384], F16).ap()
    w1b_s = nc.alloc_sbuf_tensor("w1b_s", [96, 384], F16).ap()
    wdiag_s = nc.alloc_sbuf_tensor("wdiag_s", [128, NDIAG * 128], F16).ap()
    wtap_s = nc.alloc_sbuf_tensor("wtap_s", [128, 27], F32).ap()
    ones_q = nc.alloc_sbuf_tensor("ones_q", [96, 96], F16).ap()
    dv_ones = nc.alloc_sbuf_tensor("dv_ones", [128, 48], F16).ap()
    ag_full = nc.alloc_sbuf_tensor("ag_full", [96, 256], F16).ap()
    agfs = nc.alloc_sbuf_tensor("agfs", [96, M_AG], F16).ap()
    temp_rep = nc.alloc_sbuf_tensor("temp_rep", [96, 1], F32).ap()
    as1 = nc.alloc_sbuf_tensor("as1", [96, NB * 64], F16).ap()
    t2 = nc.alloc_sbuf_tensor("t2", [96, 128], F32).ap()
    asum = nc.alloc_sbuf_tensor("asum", [96, M_AG], F32).ap()
    av_l0 = nc.alloc_sbuf_tensor("av_l0", [128, 48], F16).ap()
    av_l1 = nc.alloc_sbuf_tensor("av_l1", [128, 48], F16).ap()
    ident98 = nc.alloc_sbuf_tensor("ident98", [98, 98], F16).ap()
    mu_parts = nc.alloc_sbuf_tensor("mu_parts", [48, 4 * NB], F32).ap()
    mub = nc.alloc_sbuf_tensor("mub", [48, 2], F32).ap()
    mu_neg = nc.alloc_sbuf_tensor("mu_neg", [96, 1], F32).ap()
    mu2 = nc.alloc_sbuf_tensor("mu2", [96, 1], F32).ap()
    sq_parts = nc.alloc_sbuf_tensor("sq_parts", [96, NB], F32).ap()
    sden = nc.alloc_sbuf_tensor("sden", [96, 1], F32).ap()
    s_ch = nc.alloc_sbuf_tensor("s_ch", [96, 1], F32).ap()
    half_s = nc.alloc_sbuf_tensor("half_s", [96, 1], F32).ap()

    # aliases (sequential reuse of big buffers)
    pre3 = scratch.rearrange("p (y x) -> p y x", x=RS)
    vT3 = vT.rearrange("p (c w) -> p c w", w=128)
    x_attn = dwK[0:96, :]          # q-side output (khat dead by then)
    d2 = vT[0:96, :]               # simam squared deviations (vT dead)
    # q-side rotating slots inside dwA (a-data dead after pooling)
    e1_slots = [dwA[:, r * BLK:(r + 1) * BLK] for r in range(2)]
    xt_slots = [dwA[0:48, 4096 + r * BLK: 4096 + (r + 1) * BLK] for r in range(2)]
    rqs_slots = [dwA[0:48, 6144 + 2 * r * BLK: 6144 + 2 * (r + 1) * BLK]
                 .bitcast(F32) for r in range(2)]
    sig_slots = [dwA[0:96, 10240 + r * BLK: 10240 + (r + 1) * BLK]
                 for r in range(2)]

    with TileContext(nc) as tc:
        with (
            tc.tile_pool(name="xio", bufs=3) as xio,
            tc.tile_pool(name="wrk", bufs=2) as wrk,
            tc.tile_pool(name="ex", bufs=3) as ex,
            tc.tile_pool(name="nrm", bufs=2) as nrm,
            tc.tile_pool(name="tap", bufs=3) as tap,
            tc.tile_pool(name="pp1", bufs=2, space="PSUM") as pp1,
            tc.tile_pool(name="pp2", bufs=4, space="PSUM") as pp2,
        ):
            # ================= init =================
            nc.sync.dma_start(out=w1a_s[:], in_=w1a[:])
            nc.sync.dma_start(out=w1b_s[:], in_=w1b[:])
            nc.sync.dma_start(out=wdiag_s[:], in_=wdiag[:])
            nc.sync.dma_start(out=wtap_s[:], in_=wtap[:])
            nc.sync.dma_start(out=ones_q[:], in_=pat[0:96, 0:96])
            nc.sync.dma_start(out=dv_ones[:], in_=pat[:, 96:144])
            nc.sync.dma_start(out=ident98[:], in_=pat[0:98, 144:242])
            nc.sync.dma_start(out=temp_rep[:], in_=tmp01[:])
            nc.gpsimd.memset(ag_full[:], 0.0)
            nc.gpsimd.memset(av_l0[:], 0.0)
            nc.gpsimd.memset(av_l1[:], 0.0)
            nc.gpsimd.memset(half_s[:], 0.5)
            nc.gpsimd.memset(vT3[:, :, 15], 1.0)
            nc.gpsimd.memset(vT3[:, :, 112], 1.0)
            # pre borders (rows 0 and 129, cols 0 and 129)
            nc.gpsimd.memset(pre3[:, 0, :], 0.0)
            nc.gpsimd.memset(pre3[:, 129, :], 0.0)
            nc.gpsimd.memset(pre3[:, :, 0], 0.0)
            nc.gpsimd.memset(pre3[:, :, 129], 0.0)

            # ================= sweep1: conv1x1 + dwconv + fused norm/pool ==
            def conv_blk(s, j):
                x0 = xio.tile([96, BLK], F16, tag="x")
                x1 = xio.tile([96, BLK], F16, tag="x")
                nc.sync.dma_start(out=x0[:], in_=xin[0:96, j * BLK:(j + 1) * BLK])
                nc.sync.dma_start(out=x1[:], in_=xin[96:192, j * BLK:(j + 1) * BLK])
                ps = pp1.tile([128, BLK], F32, tag="big")
                for q in range(2):
                    sl = slice(q * 512, (q + 1) * 512)
                    nc.tensor.matmul(ps[:, sl], w1a_s[:, s * 128:(s + 1) * 128],
                                     x0[:, sl], start=True, stop=False)
                    nc.tensor.matmul(ps[:, sl], w1b_s[:, s * 128:(s + 1) * 128],
                                     x1[:, sl], start=False, stop=True)
                nc.scalar.copy(pre3[:, 1 + 8 * j: 9 + 8 * j, 1:129], ps[:])

            def dw_blk(s, j):
                dst = dws[s][:, j * BLK:(j + 1) * BLK]
                pds = [pp2.tile([128, 512], F32, tag="sml", name=f"pd{_q}")
                       for _q in range(2)]
                pe_t = PE_TAPS[s]
                for q in range(2):
                    for ti, (dy, dx) in enumerate(pe_t):
                        dg = wdiag_s[:, WDIAG_SLOT[(s, dy, dx)] * 128:
                                     (WDIAG_SLOT[(s, dy, dx)] + 1) * 128]
                        rv = pre3[:, 1 + dy + 8 * j + 4 * q: 5 + dy + 8 * j + 4 * q,
                                  1 + dx: 129 + dx]
                        nc.tensor.matmul(pds[q][:], dg, rv,
                                         start=(ti == 0), stop=(ti == len(pe_t) - 1))
                dy, dx = DVE_TAPS[s][0]
                for q in range(2):
                    rv = pre3[:, 1 + dy + 8 * j + 4 * q: 5 + dy + 8 * j + 4 * q,
                              1 + dx: 129 + dx]
                    nc.vector.scalar_tensor_tensor(
                        out=dst[:, q * 512:(q + 1) * 512], in0=rv,
                        scalar=wtap_s[:, s * 9 + _tidx(dy, dx):
                                      s * 9 + _tidx(dy, dx) + 1],
                        in1=pds[q][:], op0=OP.mult, op1=OP.add)
                # GP cannot run TensorScalarPtr: DVE makes a scaled copy,
                # GP accumulates it (plain tensor_tensor is Pool-legal).
                for dy, dx in GP_TAPS[s]:
                    rv = pre3[:, 1 + dy + 8 * j: 9 + dy + 8 * j, 1 + dx: 129 + dx]
                    tmp = tap.tile([128, BLK], F16, tag="tap")
                    nc.vector.tensor_scalar(
                        out=tmp[:], in0=rv,
                        scalar1=wtap_s[:, s * 9 + _tidx(dy, dx):
                                       s * 9 + _tidx(dy, dx) + 1],
                        scalar2=None, op0=OP.mult)
                    nc.gpsimd.tensor_tensor(out=dst, in0=dst, in1=tmp[:],
                                            op=OP.add)

            def norm_blk(t, j):
                blk = slice(j * BLK, (j + 1) * BLK)
                sq = nrm.tile([96, BLK], F16, tag="nrm")
                nc.scalar.activation(sq[:], t[0:96, blk], AF.Square)
                pn0 = pp2.tile([128, 512], F32, tag="sml")
                pn1 = pp2.tile([128, 512], F32, tag="sml")
                nc.tensor.matmul(pn0[0:96, :], ones_q[:], sq[:, 0:512],
                                 start=True, stop=True)
                nc.tensor.matmul(pn1[0:96, :], ones_q[:], sq[:, 512:1024],
                                 start=True, stop=True)
                rns = nrm.tile([96, BLK], F32, tag="rns")
                nc.vector.reciprocal_approx_fast(out=rns[:, 0:512],
                                                 in_=pn0[0:96, :])
                nc.vector.reciprocal_approx_fast(out=rns[:, 512:1024],
                                                 in_=pn1[0:96, :])
                rinv = nrm.tile([96, BLK], F16, tag="nrm")
                nc.scalar.activation(rinv[:], rns[:], AF.Sqrt)
                nc.gpsimd.tensor_tensor(out=t[0:96, blk], in0=t[0:96, blk],
                                        in1=rinv[:], op=OP.mult)

            def pool_blk(j):
                a3 = dwA[0:96, j * BLK:(j + 1) * BLK].rearrange(
                    "p (a xi) -> p a xi", xi=16)
                with nc.allow_low_precision(reason="f16 partial pool sums"):
                    nc.vector.tensor_reduce(out=as1[:, j * 64:(j + 1) * 64],
                                            in_=a3, axis=AX.X, op=OP.add)

            # slab A (a + v[0:32])
            conv_blk(0, 0)
            for j in range(1, NB):
                conv_blk(0, j)
                dw_blk(0, j - 1)
                pool_blk(j - 1)
            dw_blk(0, NB - 1)
            pool_blk(NB - 1)
            nc.sync.dma_start(out=vT3[:, :, V0:V0 + 32], in_=dwA[96:128, :],
                              transpose=True)

            # agent pooling finish: as1 col = 128r + 64jj + 8y + px
            a4 = as1.rearrange("p (rj y px) -> p rj px y", y=8, px=8)
            nc.vector.tensor_reduce(out=t2[:], in_=a4, axis=AX.X, op=OP.add)
            t24 = t2.rearrange("p (r jj px) -> p r px jj", jj=2, px=8)
            asum3 = asum.rearrange("p (r px) -> p r px", px=8)
            nc.vector.tensor_reduce(out=asum3, in_=t24, axis=AX.X, op=OP.add)
            nc.vector.tensor_scalar(out=agfs[:], in0=asum[:],
                                    scalar1=temp_rep[:], scalar2=1.0 / 256.0,
                                    op0=OP.mult, op1=OP.mult)
            for h in range(4):
                nc.sync.dma_start(
                    out=ag_full[h * 24:(h + 1) * 24, h * 64:(h + 1) * 64],
                    in_=agfs[h * 24:(h + 1) * 24, :])

            # slab K (k + v[32:64]) with fused k-norm
            conv_blk(1, 0)
            for j in range(1, NB):
                conv_blk(1, j)
                dw_blk(1, j - 1)
                norm_blk(dwK, j - 1)
            dw_blk(1, NB - 1)
            norm_blk(dwK, NB - 1)
            nc.sync.dma_start(out=vT3[:, :, V0 + 32:V0 + 64], in_=dwK[96:128, :],
                              transpose=True)

            # slab Q (q + v[64:96]) with fused q-norm
            conv_blk(2, 0)
            for j in range(1, NB):
                conv_blk(2, j)
                dw_blk(2, j - 1)
                norm_blk(dwQ, j - 1)
            dw_blk(2, NB - 1)
            norm_blk(dwQ, NB - 1)
            nc.sync.dma_start(out=vT3[:, :, V0 + 64:V0 + 96], in_=dwQ[96:128, :],
                              transpose=True)

            # ================= k-side: L2T -> exp -> agent_v =============
            # agvT[0, m] = D2 denominators, rows 1:97 = all 96 v channels.
            agvT = pp2.tile([128, 512], F32, tag="sml")
            for cc in range(NCH // 2):
                c = 2 * cc
                l2 = pp2.tile([128, 512], F32, tag="sml")
                nc.tensor.matmul(l2[:, 0:256], dwK[0:96, c * 128:(c + 1) * 128],
                                 ag_full[:], start=True, stop=True)
                nc.tensor.matmul(l2[:, 256:512],
                                 dwK[0:96, (c + 1) * 128:(c + 2) * 128],
                                 ag_full[:], start=True, stop=True)
                e2t = ex.tile([128, 512], F16, tag="e2t")
                nc.scalar.activation(e2t[:], l2[:], AF.Exp)
                nc.tensor.matmul(agvT[0:98, 0:256], vT3[:, c, 15:113],
                                 e2t[:, 0:256], start=(cc == 0), stop=False)
                nc.tensor.matmul(agvT[0:98, 0:256], vT3[:, c + 1, 15:113],
                                 e2t[:, 256:512], start=False,
                                 stop=(cc == NCH // 2 - 1))
            # transpose agvT per head-pair: trp [m, 98]; col 0 = D2.
            tv = wrk.tile([128, 256], F16, tag="tv")
            nc.scalar.copy(tv[0:98, :], agvT[0:98, 0:256])
            d2col = wrk.tile([128, 2], F32, tag="d2col")
            trps = []
            for hp in range(2):
                trp = pp1.tile([128, 128], F16, tag="big")
                nc.tensor.transpose(trp[:, 0:98], tv[0:98, hp * 128:(hp + 1) * 128],
                                    ident98[0:98, 0:98])
                nc.scalar.copy(d2col[:, hp:hp + 1], trp[:, 0:1])
                trps.append(trp)
            rqq = wrk.tile([128, 2], F32, tag="rqq")
            nc.vector.reciprocal_approx_fast(out=rqq[:], in_=d2col[:])
            # block-diagonal extraction: hp0 heads use v-ch 0:48 (cols 1:49),
            # hp1 heads use v-ch 48:96 (cols 49:97); even head -> rows 0:64 x
            # cols 0:24, odd head -> rows 64:128 x cols 24:48.
            for hp, trp, base in ((0, trps[0], 1), (1, trps[1], 49)):
                av = av_l0 if hp == 0 else av_l1
                nc.vector.tensor_scalar(out=av[0:64, 0:24],
                                        in0=trp[0:64, base:base + 24],
                                        scalar1=rqq[0:64, hp:hp + 1],
                                        scalar2=None, op0=OP.mult)
                for w0 in (64, 96):
                    nc.vector.tensor_scalar(out=av[w0:w0 + 32, 24:48],
                                            in0=trp[w0:w0 + 32,
                                                    base + 24:base + 48],
                                            scalar1=rqq[w0:w0 + 32, hp:hp + 1],
                                            scalar2=None, op0=OP.mult)

            # ================= q-side + division =========================
            for j in range(NB):
                blk = slice(j * BLK, (j + 1) * BLK)
                for hp in range(2):
                    r = hp
                    e1 = e1_slots[r]
                    l1 = pp1.tile([128, BLK], F32, tag="big")
                    for q in range(2):
                        sl = slice(j * BLK + q * 512, j * BLK + (q + 1) * 512)
                        nc.tensor.matmul(l1[:, q * 512:(q + 1) * 512],
                                         ag_full[:, hp * 128:(hp + 1) * 128],
                                         dwQ[0:96, sl], start=True, stop=True)
                    nc.scalar.activation(e1, l1[:], AF.Exp)
                    av = av_l0 if hp == 0 else av_l1
                    ops = [pp2.tile([128, 512], F32, tag="sml", name=f"op{_q}")
                           for _q in range(2)]
                    ods = [pp2.tile([128, 512], F32, tag="sml", name=f"od{_q}")
                           for _q in range(2)]
                    for q in range(2):
                        psl = slice(q * 512, (q + 1) * 512)
                        nc.tensor.matmul(ops[q][0:48, :], av[:], e1[:, psl],
                                         start=True, stop=True)
                        nc.tensor.matmul(ods[q][0:48, :], dv_ones[:], e1[:, psl],
                                         start=True, stop=True)
                    rqs = rqs_slots[r]
                    xt = xt_slots[r]
                    for q in range(2):
                        qsl = slice(q * 512, (q + 1) * 512)
                        nc.vector.reciprocal_approx_fast(out=rqs[:, qsl],
                                                         in_=ods[q][0:48, :])
                        mcol = 4 * j + 2 * hp + q
                        dst_half = (x_attn[0:48, j * BLK + q * 512:
                                           j * BLK + (q + 1) * 512]
                                    if hp == 0 else xt[:, qsl])
                        nc.vector.scalar_tensor_tensor(
                            out=dst_half, in0=ops[q][0:48, :], scalar=0.0,
                            in1=rqs[:, qsl], op0=OP.bypass, op1=OP.mult,
                            accum_out=mu_parts[:, mcol:mcol + 1])
                    if hp == 1:
                        nc.sync.dma_start(out=dwK[48:96, blk], in_=xt)
                # raw second moment per block: sden = sum x^2 - N*mu^2
                nc.scalar.activation(d2[:, blk], x_attn[:, blk], AF.Square,
                                     accum_out=sq_parts[:, j:j + 1])

            # ================= SimAM =====================================
            mp4 = mu_parts.rearrange("p (j c q) -> p c j q", c=2, q=2)
            nc.vector.tensor_reduce(out=mub[:], in_=mp4, axis=AX.XY, op=OP.add)
            nc.vector.tensor_scalar(out=mub[:], in0=mub[:],
                                    scalar1=-1.0 / N, scalar2=None, op0=OP.mult)
            nc.sync.dma_start(out=mu_neg[0:48, :], in_=mub[:, 0:1])
            nc.sync.dma_start(out=mu_neg[48:96, :], in_=mub[:, 1:2])
            nc.vector.reduce_sum(sden[:], sq_parts[:], axis=AX.X)
            nc.vector.tensor_tensor(out=mu2[:], in0=mu_neg[:], in1=mu_neg[:],
                                    op=OP.mult)
            nc.vector.scalar_tensor_tensor(out=sden[:], in0=mu2[:],
                                           scalar=-float(N), in1=sden[:],
                                           op0=OP.mult, op1=OP.add)
            nc.vector.tensor_scalar(out=sden[:], in0=sden[:],
                                    scalar1=4.0 / (N - 1), scalar2=4.0 * E_LAMBDA,
                                    op0=OP.mult, op1=OP.add)
            nc.vector.reciprocal_approx_fast(out=s_ch[:], in_=sden[:])
            for j in range(NB):
                blk = slice(j * BLK, (j + 1) * BLK)
                nc.scalar.activation(d2[:, blk], x_attn[:, blk], AF.Square,
                                     bias=mu_neg[:], scale=1.0)
                sig = sig_slots[j % 2]
                nc.scalar.activation(sig, d2[:, blk], AF.Sigmoid,
                                     bias=half_s[:], scale=s_ch[:])
                nc.vector.tensor_tensor(out=x_attn[:, blk], in0=x_attn[:, blk],
                                        in1=sig, op=OP.mult)
                nc.sync.dma_start(out=out_d[:, blk], in_=x_attn[:, blk])

    nc.compile()
    return nc


_NC = None


def _get_nc():
    global _NC
    if _NC is None:
        _install_ntff_hook()
        _NC = build_nc()
    return _NC


def make_core_inputs(x, w_qkv, w_dw, temperature):
    """Host-side shard prep. Returns list of 8 input dicts."""
    x = np.asarray(x)
    w_qkv = np.asarray(w_qkv)
    w_dw = np.asarray(w_dw)
    temperature = np.asarray(temperature).reshape(8)
    in_maps = []
    for core in range(8):
        b, hg = core // 2, core % 2
        qr = hg * 96 + np.arange(96)
        kr = 192 + hg * 96 + np.arange(96)
        vr = 384 + hg * 96 + np.arange(96)
        ar = 576 + hg * 96 + np.arange(96)
        rows = np.concatenate([
            ar, vr[0:32],       # slab A
            kr, vr[32:64],      # slab K
            qr, vr[64:96],      # slab Q
        ])
        W1 = w_qkv[rows, :, 0, 0]                        # [384, 192]
        W1T = np.ascontiguousarray(W1.T).astype(np.float16)
        wd9 = w_dw[rows, 0].reshape(384, 9).astype(np.float32)
        wdiag_h = np.zeros((128, NDIAG * 128), np.float16)
        wtap_h = np.zeros((128, 27), np.float32)
        for s in range(3):
            for t in range(9):
                wtap_h[:, s * 9 + t] = wd9[s * 128:(s + 1) * 128, t]
        for (s, dy, dx), idx in WDIAG_SLOT.items():
            t = _tidx(dy, dx)
            wdiag_h[np.arange(128), idx * 128 + np.arange(128)] = \
                wd9[s * 128:(s + 1) * 128, t].astype(np.float16)
        pat_h = np.zeros((128, 242), np.float16)
        for h in range(4):
            pat_h[h * 24:(h + 1) * 24, h * 24:(h + 1) * 24] = 1    # ones_q
        pat_h[0:64, 96:120] = 1                                    # dv_ones
        pat_h[64:128, 120:144] = 1
        pat_h[np.arange(98), 144 + np.arange(98)] = 1              # ident98
        heads = np.arange(hg * 4, hg * 4 + 4)
        t4 = temperature[heads].astype(np.float32)
        in_maps.append({
            "xin": x[b].reshape(192, N).astype(np.float16),
            "w1a": W1T[0:96].copy(),
            "w1b": W1T[96:192].copy(),
            "wdiag": wdiag_h,
            "wtap": wtap_h,
            "tmp01": np.repeat(t4, 24).reshape(96, 1).copy(),
            "pat": pat_h,
        })
    return in_maps


def _assemble(results):
    full = np.empty((B, C, H, W), np.float32)
    for core in range(8):
        b, hg = core // 2, core % 2
        full[b, hg * 96:(hg + 1) * 96] = \
            results[core]["out"].astype(np.float32).reshape(96, H, W)
    return full


def kernel(x, w_qkv, w_dw, temperature):
    nc = _get_nc()
    in_maps = make_core_inputs(x, w_qkv, w_dw, temperature)
    res = run_bass_kernel_spmd(nc, in_maps, list(range(8)))
    return _assemble(res.results)


def kernel_profiled(x, w_qkv, w_dw, temperature):
    nc = _get_nc()
    in_maps = make_core_inputs(x, w_qkv, w_dw, temperature)
    res = run_bass_kernel_spmd(nc, in_maps, list(range(8)), trace=True)
    return _assemble(res.results), res.exec_time_ns


# revision 38
# speedup vs baseline: 1.1787x; 1.1787x over previous
"""MASA agent-attention kernel for Trainium2, 8-core SPMD.

Sharding: core = (batch b in 0..3) x (head-group hg in 0..1); each core
computes 4 heads (96 of 192 output channels) for one batch image.
No cross-core communication.

Per-core channel slabs of 128 (processed in this order):
  slab 0 "A": rows 0:96 = a-channels, rows 96:128 = v[0:32]
  slab 1 "K": rows 0:96 = k-channels, rows 96:128 = v[32:64]
  slab 2 "Q": rows 0:96 = q-channels, rows 96:128 = v[64:96]
so agent pooling (a) and the k-norm can start as early as possible, and
both norms use the same block-diagonal ones matrix at partition base 0.

v is transposed to [spatial, channel] with 3 large xbar-transpose DMAs
(out AP [128, chunk, ch] => out[p, c, ch] = v[ch, c*128+p]), replacing
the per-chunk transpose DMAs.

Engine-op partition windows must be 32-aligned and (base==0 or count<=32).
"""

import sys
import types
import numpy as np

import concourse.bacc as bacc
import concourse.bass as bass
import concourse.mybir as mybir
from concourse.tile import TileContext
from concourse.bass_utils import run_bass_kernel_spmd

F16 = mybir.dt.float16
F32 = mybir.dt.float32
AX = mybir.AxisListType
OP = mybir.AluOpType
AF = mybir.ActivationFunctionType

B, C, H, W = 4, 192, 128, 128
N = H * W              # 16384
M_AG = 64              # agent tokens
E_LAMBDA = 1e-4
RS = 130               # padded row stride for pre
PREFREE = RS * RS      # 16900
NB = 16                # blocks of 1024 spatial positions
BLK = 1024
NCH = 128              # k-side chunks of 128

# dwconv 3x3 tap split: PE does diag-matmul taps, DVE/GpSimd do
# scalar*tensor+tensor taps (dx=+-1 keeps their inner APs 4B-aligned).
# Slab A gives DVE only the psum-merge tap: DVE also runs the pooling
# reductions there (GpSimd cannot reduce along the free axis).
PE_TAPS = {s: [(-1, -1), (-1, 0), (-1, 1), (0, 0), (1, -1), (1, 0), (1, 1)]
           for s in range(3)}
DVE_TAPS = {0: [(0, -1)],           # first one also merges the PE psum
            1: [(0, -1)],
            2: [(0, -1)]}
GP_TAPS = {0: [(0, 1)],
           1: [(0, 1)],
           2: [(0, 1)]}
WDIAG_SLOT = {}
for _s in range(3):
    for _t in PE_TAPS[_s]:
        WDIAG_SLOT[(_s, _t[0], _t[1])] = len(WDIAG_SLOT)
NDIAG = len(WDIAG_SLOT)

# vT chunk layout (width 128): col 15 = 1, cols 16:112 = v[0:96], col 112 = 1
V0 = 16


def _tidx(dy, dx):
    return (dy + 1) * 3 + (dx + 1)


def _install_ntff_hook():
    try:
        import antenv.axon_hooks  # noqa: F401
        return
    except ImportError:
        pass
    try:
        from trn_agent_boot.trn_boot import _ntff_profile_via_ctypes
        hook = _ntff_profile_via_ctypes('/opt/axon/libaxon_pjrt.so')
        mod = types.ModuleType("antenv.axon_hooks")
        mod.get_axon_ntff_profile_hook = lambda: hook
        mod.set_axon_ntff_profile_hook = lambda h: None
        sys.modules["antenv.axon_hooks"] = mod
    except Exception:
        pass


def build_nc():
    nc = bacc.Bacc("TRN2", target_bir_lowering=False, debug=False, num_devices=8)

    # ---- DRAM I/O ----
    xin = nc.dram_tensor("xin", [192, N], F16, kind="ExternalInput").ap()
    w1a = nc.dram_tensor("w1a", [96, 384], F16, kind="ExternalInput").ap()
    w1b = nc.dram_tensor("w1b", [96, 384], F16, kind="ExternalInput").ap()
    wdiag = nc.dram_tensor("wdiag", [128, NDIAG * 128], F16, kind="ExternalInput").ap()
    wtap = nc.dram_tensor("wtap", [128, 27], F32, kind="ExternalInput").ap()
    tmp01 = nc.dram_tensor("tmp01", [96, 1], F32, kind="ExternalInput").ap()
    pat = nc.dram_tensor("pat", [128, 242], F16, kind="ExternalInput").ap()
    out_d = nc.dram_tensor("out", [96, N], F16, kind="ExternalOutput").ap()

    # ---- persistent SBUF ----
    scratch = nc.alloc_sbuf_tensor("scratch", [128, PREFREE], F16).ap()
    dwA = nc.alloc_sbuf_tensor("dwA", [128, N], F16).ap()
    dwK = nc.alloc_sbuf_tensor("dwK", [128, N], F16).ap()
    dwQ = nc.alloc_sbuf_tensor("dwQ", [128, N], F16).ap()
    vT = nc.alloc_sbuf_tensor("vT", [128, N], F16).ap()
    dws = [dwA, dwK, dwQ]
    w1a_s = nc.alloc_sbuf_tensor("w1a_s", [96, 384], F16).ap()
    w1b_s = nc.alloc_sbuf_tensor("w1b_s", [96, 384], F16).ap()
    wdiag_s = nc.alloc_sbuf_tensor("wdiag_s", [128, NDIAG * 128], F16).ap()
    wtap_s = nc.alloc_sbuf_tensor("wtap_s", [128, 27], F32).ap()
    ones_q = nc.alloc_sbuf_tensor("ones_q", [96, 96], F16).ap()
    dv_ones = nc.alloc_sbuf_tensor("dv_ones", [128, 48], F16).ap()
    ag_full = nc.alloc_sbuf_tensor("ag_full", [96, 256], F16).ap()
    agfs = nc.alloc_sbuf_tensor("agfs", [96, M_AG], F16).ap()
    temp_rep = nc.alloc_sbuf_tensor("temp_rep", [96, 1], F32).ap()
    as1 = nc.alloc_sbuf_tensor("as1", [96, NB * 64], F16).ap()
    t2 = nc.alloc_sbuf_tensor("t2", [96, 128], F32).ap()
    asum = nc.alloc_sbuf_tensor("asum", [96, M_AG], F32).ap()
    av_l0 = nc.alloc_sbuf_tensor("av_l0", [128, 48], F16).ap()
    av_l1 = nc.alloc_sbuf_tensor("av_l1", [128, 48], F16).ap()
    ident98 = nc.alloc_sbuf_tensor("ident98", [98, 98], F16).ap()
    mu_parts = nc.alloc_sbuf_tensor("mu_parts", [48, 4 * NB], F32).ap()
    mub = nc.alloc_sbuf_tensor("mub", [48, 2], F32).ap()
    mu_neg = nc.alloc_sbuf_tensor("mu_neg", [96, 1], F32).ap()
    mu2 = nc.alloc_sbuf_tensor("mu2", [96, 1], F32).ap()
    sq_parts = nc.alloc_sbuf_tensor("sq_parts", [96, NB], F32).ap()
    sden = nc.alloc_sbuf_tensor("sden", [96, 1], F32).ap()
    s_ch = nc.alloc_sbuf_tensor("s_ch", [96, 1], F32).ap()
    half_s = nc.alloc_sbuf_tensor("half_s", [96, 1], F32).ap()

    # aliases (sequential reuse of big buffers)
    pre3 = scratch.rearrange("p (y x) -> p y x", x=RS)
    vT3 = vT.rearrange("p (c w) -> p c w", w=128)
    x_attn = dwK[0:96, :]          # q-side output (khat dead by then)
    d2 = vT[0:96, :]               # simam squared deviations (vT dead)
    # q-side rotating slots inside dwA (a-data dead after pooling)
    e1_slots = [dwA[:, r * BLK:(r + 1) * BLK] for r in range(2)]
    xt_slots = [dwA[0:48, 4096 + r * BLK: 4096 + (r + 1) * BLK] for r in range(2)]
    rqs_slots = [dwA[0:48, 6144 + 2 * r * BLK: 6144 + 2 * (r + 1) * BLK]
                 .bitcast(F32) for r in range(2)]
    sig_slots = [dwA[0:96, 10240 + r * BLK: 10240 + (r + 1) * BLK]
                 for r in range(2)]

    with TileContext(nc) as tc:
        with (
            tc.tile_pool(name="xio", bufs=3) as xio,
            tc.tile_pool(name="wrk", bufs=2) as wrk,
            tc.tile_pool(name="ex", bufs=3) as ex,
            tc.tile_pool(name="nrm", bufs=2) as nrm,
            tc.tile_pool(name="tap", bufs=3) as tap,
            tc.tile_pool(name="pp1", bufs=3, space="PSUM") as pp1,
            tc.tile_pool(name="pp2", bufs=2, space="PSUM") as pp2,
        ):
            # ================= init =================
            nc.sync.dma_start(out=w1a_s[:], in_=w1a[:])
            nc.sync.dma_start(out=w1b_s[:], in_=w1b[:])
            nc.sync.dma_start(out=wdiag_s[:], in_=wdiag[:])
            nc.sync.dma_start(out=wtap_s[:], in_=wtap[:])
            nc.sync.dma_start(out=ones_q[:], in_=pat[0:96, 0:96])
            nc.sync.dma_start(out=dv_ones[:], in_=pat[:, 96:144])
            nc.sync.dma_start(out=ident98[:], in_=pat[0:98, 144:242])
            nc.sync.dma_start(out=temp_rep[:], in_=tmp01[:])
            nc.gpsimd.memset(ag_full[:], 0.0)
            nc.gpsimd.memset(av_l0[:], 0.0)
            nc.gpsimd.memset(av_l1[:], 0.0)
            nc.gpsimd.memset(half_s[:], 0.5)
            nc.gpsimd.memset(vT3[:, :, 15], 1.0)
            nc.gpsimd.memset(vT3[:, :, 112], 1.0)
            # pre borders (rows 0 and 129, cols 0 and 129)
            nc.gpsimd.memset(pre3[:, 0, :], 0.0)
            nc.gpsimd.memset(pre3[:, 129, :], 0.0)
            nc.gpsimd.memset(pre3[:, :, 0], 0.0)
            nc.gpsimd.memset(pre3[:, :, 129], 0.0)

            # ================= sweep1: conv1x1 + dwconv + fused norm/pool ==
            def conv_blk(s, j):
                x0 = xio.tile([96, BLK], F16, tag="x")
                x1 = xio.tile([96, BLK], F16, tag="x")
                nc.sync.dma_start(out=x0[:], in_=xin[0:96, j * BLK:(j + 1) * BLK])
                nc.sync.dma_start(out=x1[:], in_=xin[96:192, j * BLK:(j + 1) * BLK])
                ps = pp1.tile([128, BLK], F32, tag="big")
                for q in range(2):
                    sl = slice(q * 512, (q + 1) * 512)
                    nc.tensor.matmul(ps[:, sl], w1a_s[:, s * 128:(s + 1) * 128],
                                     x0[:, sl], start=True, stop=False)
                    nc.tensor.matmul(ps[:, sl], w1b_s[:, s * 128:(s + 1) * 128],
                                     x1[:, sl], start=False, stop=True)
                nc.scalar.copy(pre3[:, 1 + 8 * j: 9 + 8 * j, 1:129], ps[:])

            def dw_blk(s, j):
                dst = dws[s][:, j * BLK:(j + 1) * BLK]
                pd = pp1.tile([128, BLK], F32, tag="big")
                pe_t = PE_TAPS[s]
                for q in range(2):
                    for ti, (dy, dx) in enumerate(pe_t):
                        dg = wdiag_s[:, WDIAG_SLOT[(s, dy, dx)] * 128:
                                     (WDIAG_SLOT[(s, dy, dx)] + 1) * 128]
                        rv = pre3[:, 1 + dy + 8 * j + 4 * q: 5 + dy + 8 * j + 4 * q,
                                  1 + dx: 129 + dx]
                        nc.tensor.matmul(pd[:, q * 512:(q + 1) * 512], dg, rv,
                                         start=(ti == 0), stop=(ti == len(pe_t) - 1))
                dy, dx = DVE_TAPS[s][0]
                for q in range(2):
                    rv = pre3[:, 1 + dy + 8 * j + 4 * q: 5 + dy + 8 * j + 4 * q,
                              1 + dx: 129 + dx]
                    nc.vector.scalar_tensor_tensor(
                        out=dst[:, q * 512:(q + 1) * 512], in0=rv,
                        scalar=wtap_s[:, s * 9 + _tidx(dy, dx):
                                      s * 9 + _tidx(dy, dx) + 1],
                        in1=pd[:, q * 512:(q + 1) * 512], op0=OP.mult, op1=OP.add)
                # GP cannot run TensorScalarPtr: DVE makes a scaled copy,
                # GP accumulates it (plain tensor_tensor is Pool-legal).
                for dy, dx in GP_TAPS[s]:
                    rv = pre3[:, 1 + dy + 8 * j: 9 + dy + 8 * j, 1 + dx: 129 + dx]
                    tmp = tap.tile([128, BLK], F16, tag="tap")
                    nc.vector.tensor_scalar(
                        out=tmp[:], in0=rv,
                        scalar1=wtap_s[:, s * 9 + _tidx(dy, dx):
                                       s * 9 + _tidx(dy, dx) + 1],
                        scalar2=None, op0=OP.mult)
                    nc.gpsimd.tensor_tensor(out=dst, in0=dst, in1=tmp[:],
                                            op=OP.add)

            def norm_blk(t, j):
                blk = slice(j * BLK, (j + 1) * BLK)
                sq = nrm.tile([96, BLK], F16, tag="nrm")
                nc.scalar.activation(sq[:], t[0:96, blk], AF.Square)
                pn0 = pp2.tile([128, 512], F32, tag="sml")
                pn1 = pp2.tile([128, 512], F32, tag="sml")
                nc.tensor.matmul(pn0[0:96, :], ones_q[:], sq[:, 0:512],
                                 start=True, stop=True)
                nc.tensor.matmul(pn1[0:96, :], ones_q[:], sq[:, 512:1024],
                                 start=True, stop=True)
                rns = nrm.tile([96, BLK], F32, tag="rns")
                nc.vector.reciprocal_approx_fast(out=rns[:, 0:512],
                                                 in_=pn0[0:96, :])
                nc.vector.reciprocal_approx_fast(out=rns[:, 512:1024],
                                                 in_=pn1[0:96, :])
                rinv = nrm.tile([96, BLK], F16, tag="nrm")
                nc.scalar.activation(rinv[:], rns[:], AF.Sqrt)
                nc.gpsimd.tensor_tensor(out=t[0:96, blk], in0=t[0:96, blk],
                                        in1=rinv[:], op=OP.mult)

            def pool_blk(j):
                a3 = dwA[0:96, j * BLK:(j + 1) * BLK].rearrange(
                    "p (a xi) -> p a xi", xi=16)
                with nc.allow_low_precision(reason="f16 partial pool sums"):
                    nc.vector.tensor_reduce(out=as1[:, j * 64:(j + 1) * 64],
                                            in_=a3, axis=AX.X, op=OP.add)

            # slab A (a + v[0:32])
            conv_blk(0, 0)
            for j in range(1, NB):
                conv_blk(0, j)
                dw_blk(0, j - 1)
                pool_blk(j - 1)
            dw_blk(0, NB - 1)
            pool_blk(NB - 1)
            nc.sync.dma_start(out=vT3[:, :, V0:V0 + 32], in_=dwA[96:128, :],
                              transpose=True)

            # agent pooling finish: as1 col = 128r + 64jj + 8y + px
            a4 = as1.rearrange("p (rj y px) -> p rj px y", y=8, px=8)
            nc.vector.tensor_reduce(out=t2[:], in_=a4, axis=AX.X, op=OP.add)
            t24 = t2.rearrange("p (r jj px) -> p r px jj", jj=2, px=8)
            asum3 = asum.rearrange("p (r px) -> p r px", px=8)
            nc.vector.tensor_reduce(out=asum3, in_=t24, axis=AX.X, op=OP.add)
            nc.vector.tensor_scalar(out=agfs[:], in0=asum[:],
                                    scalar1=temp_rep[:], scalar2=1.0 / 256.0,
                                    op0=OP.mult, op1=OP.mult)
            for h in range(4):
                nc.sync.dma_start(
                    out=ag_full[h * 24:(h + 1) * 24, h * 64:(h + 1) * 64],
                    in_=agfs[h * 24:(h + 1) * 24, :])

            # slab K (k + v[32:64]) with fused k-norm
            conv_blk(1, 0)
            for j in range(1, NB):
                conv_blk(1, j)
                dw_blk(1, j - 1)
                norm_blk(dwK, j - 1)
            dw_blk(1, NB - 1)
            norm_blk(dwK, NB - 1)
            nc.sync.dma_start(out=vT3[:, :, V0 + 32:V0 + 64], in_=dwK[96:128, :],
                              transpose=True)

            # slab Q (q + v[64:96]) with fused q-norm
            conv_blk(2, 0)
            for j in range(1, NB):
                conv_blk(2, j)
                dw_blk(2, j - 1)
                norm_blk(dwQ, j - 1)
            dw_blk(2, NB - 1)
            norm_blk(dwQ, NB - 1)
            nc.sync.dma_start(out=vT3[:, :, V0 + 64:V0 + 96], in_=dwQ[96:128, :],
                              transpose=True)

            # ================= k-side: L2T -> exp -> agent_v =============
            # agvT[0, m] = D2 denominators, rows 1:97 = all 96 v channels.
            agvT = pp2.tile([128, 512], F32, tag="sml")
            for cc in range(NCH // 2):
                c = 2 * cc
                l2 = pp1.tile([128, 512], F32, tag="big")
                nc.tensor.matmul(l2[:, 0:256], dwK[0:96, c * 128:(c + 1) * 128],
                                 ag_full[:], start=True, stop=True)
                nc.tensor.matmul(l2[:, 256:512],
                                 dwK[0:96, (c + 1) * 128:(c + 2) * 128],
                                 ag_full[:], start=True, stop=True)
                e2t = ex.tile([128, 512], F16, tag="e2t")
                nc.scalar.activation(e2t[:], l2[:], AF.Exp)
                nc.tensor.matmul(agvT[0:98, 0:256], vT3[:, c, 15:113],
                                 e2t[:, 0:256], start=(cc == 0), stop=False)
                nc.tensor.matmul(agvT[0:98, 0:256], vT3[:, c + 1, 15:113],
                                 e2t[:, 256:512], start=False,
                                 stop=(cc == NCH // 2 - 1))
            # transpose agvT per head-pair: trp [m, 98]; col 0 = D2.
            tv = wrk.tile([128, 256], F16, tag="tv")
            nc.scalar.copy(tv[0:98, :], agvT[0:98, 0:256])
            d2col = wrk.tile([128, 2], F32, tag="d2col")
            trps = []
            for hp in range(2):
                trp = pp1.tile([128, 128], F16, tag="big")
                nc.tensor.transpose(trp[:, 0:98], tv[0:98, hp * 128:(hp + 1) * 128],
                                    ident98[0:98, 0:98])
                nc.scalar.copy(d2col[:, hp:hp + 1], trp[:, 0:1])
                trps.append(trp)
            rqq = wrk.tile([128, 2], F32, tag="rqq")
            nc.vector.reciprocal_approx_fast(out=rqq[:], in_=d2col[:])
            # block-diagonal extraction: hp0 heads use v-ch 0:48 (cols 1:49),
            # hp1 heads use v-ch 48:96 (cols 49:97); even head -> rows 0:64 x
            # cols 0:24, odd head -> rows 64:128 x cols 24:48.
            for hp, trp, base in ((0, trps[0], 1), (1, trps[1], 49)):
                av = av_l0 if hp == 0 else av_l1
                nc.vector.tensor_scalar(out=av[0:64, 0:24],
                                        in0=trp[0:64, base:base + 24],
                                        scalar1=rqq[0:64, hp:hp + 1],
                                        scalar2=None, op0=OP.mult)
                for w0 in (64, 96):
                    nc.vector.tensor_scalar(out=av[w0:w0 + 32, 24:48],
                                            in0=trp[w0:w0 + 32,
                                                    base + 24:base + 48],
                                            scalar1=rqq[w0:w0 + 32, hp:hp + 1],
                                            scalar2=None, op0=OP.mult)

            # ================= q-side + division =========================
            for j in range(NB):
                blk = slice(j * BLK, (j + 1) * BLK)
                for hp in range(2):
                    r = hp
                    e1 = e1_slots[r]
                    l1 = pp1.tile([128, BLK], F32, tag="big")
                    for q in range(2):
                        sl = slice(j * BLK + q * 512, j * BLK + (q + 1) * 512)
                        nc.tensor.matmul(l1[:, q * 512:(q + 1) * 512],
                                         ag_full[:, hp * 128:(hp + 1) * 128],
                                         dwQ[0:96, sl], start=True, stop=True)
                    nc.scalar.activation(e1, l1[:], AF.Exp)
                    av = av_l0 if hp == 0 else av_l1
                    op_ = pp1.tile([128, BLK], F32, tag="big")
                    ods = [pp2.tile([128, 512], F32, tag="sml", name=f"od{_q}")
                           for _q in range(2)]
                    for q in range(2):
                        psl = slice(q * 512, (q + 1) * 512)
                        nc.tensor.matmul(op_[0:48, psl], av[:], e1[:, psl],
                                         start=True, stop=True)
                        nc.tensor.matmul(ods[q][0:48, :], dv_ones[:], e1[:, psl],
                                         start=True, stop=True)
                    rqs = rqs_slots[r]
                    xt = xt_slots[r]
                    for q in range(2):
                        qsl = slice(q * 512, (q + 1) * 512)
                        nc.vector.reciprocal_approx_fast(out=rqs[:, qsl],
                                                         in_=ods[q][0:48, :])
                        mcol = 4 * j + 2 * hp + q
                        dst_half = (x_attn[0:48, j * BLK + q * 512:
                                           j * BLK + (q + 1) * 512]
                                    if hp == 0 else xt[:, qsl])
                        nc.vector.scalar_tensor_tensor(
                            out=dst_half, in0=op_[0:48, qsl], scalar=0.0,
                            in1=rqs[:, qsl], op0=OP.bypass, op1=OP.mult,
                            accum_out=mu_parts[:, mcol:mcol + 1])
                    if hp == 1:
                        nc.sync.dma_start(out=dwK[48:96, blk], in_=xt)
                # raw second moment per block: sden = sum x^2 - N*mu^2
                nc.scalar.activation(d2[:, blk], x_attn[:, blk], AF.Square,
                                     accum_out=sq_parts[:, j:j + 1])

            # ================= SimAM =====================================
            mp4 = mu_parts.rearrange("p (j c q) -> p c j q", c=2, q=2)
            nc.vector.tensor_reduce(out=mub[:], in_=mp4, axis=AX.XY, op=OP.add)
            nc.vector.tensor_scalar(out=mub[:], in0=mub[:],
                                    scalar1=-1.0 / N, scalar2=None, op0=OP.mult)
            nc.sync.dma_start(out=mu_neg[0:48, :], in_=mub[:, 0:1])
            nc.sync.dma_start(out=mu_neg[48:96, :], in_=mub[:, 1:2])
            nc.vector.reduce_sum(sden[:], sq_parts[:], axis=AX.X)
            nc.vector.tensor_tensor(out=mu2[:], in0=mu_neg[:], in1=mu_neg[:],
                                    op=OP.mult)
            nc.vector.scalar_tensor_tensor(out=sden[:], in0=mu2[:],
                                           scalar=-float(N), in1=sden[:],
                                           op0=OP.mult, op1=OP.add)
            nc.vector.tensor_scalar(out=sden[:], in0=sden[:],
                                    scalar1=4.0 / (N - 1), scalar2=4.0 * E_LAMBDA,
                                    op0=OP.mult, op1=OP.add)
            nc.vector.reciprocal_approx_fast(out=s_ch[:], in_=sden[:])
            for j in range(NB):
                blk = slice(j * BLK, (j + 1) * BLK)
                nc.scalar.activation(d2[:, blk], x_attn[:, blk], AF.Square,
                                     bias=mu_neg[:], scale=1.0)
                sig = sig_slots[j % 2]
                nc.scalar.activation(sig, d2[:, blk], AF.Sigmoid,
                                     bias=half_s[:], scale=s_ch[:])
                nc.vector.tensor_tensor(out=x_attn[:, blk], in0=x_attn[:, blk],
                                        in1=sig, op=OP.mult)
                nc.sync.dma_start(out=out_d[:, blk], in_=x_attn[:, blk])

    nc.compile()
    return nc


_NC = None


def _get_nc():
    global _NC
    if _NC is None:
        _install_ntff_hook()
        _NC = build_nc()
    return _NC


def make_core_inputs(x, w_qkv, w_dw, temperature):
    """Host-side shard prep. Returns list of 8 input dicts."""
    x = np.asarray(x)
    w_qkv = np.asarray(w_qkv)
    w_dw = np.asarray(w_dw)
    temperature = np.asarray(temperature).reshape(8)
    in_maps = []
    for core in range(8):
        b, hg = core // 2, core % 2
        qr = hg * 96 + np.arange(96)
        kr = 192 + hg * 96 + np.arange(96)
        vr = 384 + hg * 96 + np.arange(96)
        ar = 576 + hg * 96 + np.arange(96)
        rows = np.concatenate([
            ar, vr[0:32],       # slab A
            kr, vr[32:64],      # slab K
            qr, vr[64:96],      # slab Q
        ])
        W1 = w_qkv[rows, :, 0, 0]                        # [384, 192]
        W1T = np.ascontiguousarray(W1.T).astype(np.float16)
        wd9 = w_dw[rows, 0].reshape(384, 9).astype(np.float32)
        wdiag_h = np.zeros((128, NDIAG * 128), np.float16)
        wtap_h = np.zeros((128, 27), np.float32)
        for s in range(3):
            for t in range(9):
                wtap_h[:, s * 9 + t] = wd9[s * 128:(s + 1) * 128, t]
        for (s, dy, dx), idx in WDIAG_SLOT.items():
            t = _tidx(dy, dx)
            wdiag_h[np.arange(128), idx * 128 + np.arange(128)] = \
                wd9[s * 128:(s + 1) * 128, t].astype(np.float16)
        pat_h = np.zeros((128, 242), np.float16)
        for h in range(4):
            pat_h[h * 24:(h + 1) * 24, h * 24:(h + 1) * 24] = 1    # ones_q
        pat_h[0:64, 96:120] = 1                                    # dv_ones
        pat_h[64:128, 120:144] = 1
        pat_h[np.arange(98), 144 + np.arange(98)] = 1              # ident98
        heads = np.arange(hg * 4, hg * 4 + 4)
        t4 = temperature[heads].astype(np.float32)
        in_maps.append({
            "xin": x[b].reshape(192, N).astype(np.float16),
            "w1a": W1T[0:96].copy(),
            "w1b": W1T[96:192].copy(),
            "wdiag": wdiag_h,
            "wtap": wtap_h,
            "tmp01": np.repeat(t4, 24).reshape(96, 1).copy(),
            "pat": pat_h,
        })
    return in_maps


def _assemble(results):
    full = np.empty((B, C, H, W), np.float32)
    for core in range(8):
        b, hg = core // 2, core % 2
        full[b, hg * 96:(hg + 1) * 96] = \
            results[core]["out"].astype(np.float32).reshape(96, H, W)
    return full


def kernel(x, w_qkv, w_dw, temperature):
    nc = _get_nc()
    in_maps = make_core_inputs(x, w_qkv, w_dw, temperature)
    res = run_bass_kernel_spmd(nc, in_maps, list(range(8)))
    return _assemble(res.results)


def kernel_profiled(x, w_qkv, w_dw, temperature):
    nc = _get_nc()
    in_maps = make_core_inputs(x, w_qkv, w_dw, temperature)
    res = run_bass_kernel_spmd(nc, in_maps, list(range(8)), trace=True)
    return _assemble(res.results), res.exec_time_ns
